# revision 5
# baseline (speedup 1.0000x reference)
"""MoE expert FFN (grouped GEMM) Trainium2 kernel, fp8-hybrid GEMM1.

Problem: inputs [W=8, E=4, C=2048, H=1024] fp32, per-expert FFN
(W1 [E,H,4F], b1, W2 [E,4F,H], b2) with tanh-approx GELU between.
out[w,e,c,:] = FFN_e(inputs[w,e,c,:]).

Sharding (expert-parallel x token-parallel, 8 cores): core c handles
expert e = c//2 and world-slice w in [0,4) or [4,8) by c%2 -> 8192
tokens per core, one expert's weights per core.

The baseline bf16 kernel sits at 97.4% of the bf16 PE roofline
(1 cycle/row), so the only remaining lever is fp8 DoubleRow (2 fp8
weights/PE cell, ~1.44x measured at FD=512). Full fp8 fails the 2e-2
rel-err gate (measured 5.1e-2), so GEMM1 runs a hybrid: k-tiles 0,1
of the H contraction (256 of 1024) are e4m3 and fused into ONE
DoubleRow matmul per m-tile; k-tiles 2..7 stay bf16. Reciprocal
power-of-2 scales (X*2^-2, W1*2^2 -- exact, exponent-only) minimize
quantization noise while keeping the PSUM accumulation scale-free.
Predicted rel err 1.87e-2 (vs 3.2e-3 bf16-only), saves ~2 matmul
slots -> ~1 DR slot per m-tile in GEMM1 (~70us).

Device layout (contraction dim always on SBUF partitions):
  x8   [128, 2, T]      e4m3   x8[p,i,t]    = X[t, i*128+p] * 0.25
  xt   [128, 6, T]      bf16   xt[p,k,t]    = X[t, (k+2)*128+p]
  w1dr [128, 32, 2,128] e4m3   w1dr[p,m,i,c]= W1[i*128+p, m*128+c] * 4
  w1   [128, 32, 6,128] bf16   w1[p,m,k,c]  = W1[(k+2)*128+p, m*128+c]
  w2   [128, 8, 32,128] bf16   w2[p,m,k,c]  = W2[k*128+p, m*128+c]
  b1   [128, 32]        f32    b1[p,m]      = b1_full[m*128+p]
  b2   [128, 8]         f32    b2[p,m]      = b2_full[m*128+p]
  out  [128, 8, T]      f32    out[p,m,t]   = Y[t, m*128+p]

Per 512-token chunk: GEMM1 accumulates 1 DoubleRow + 6 bf16 k-tiles
into a PSUM bank per dff-tile (32 of them), ACT applies bias+gelu
PSUM->SBUF bf16, GEMM2 accumulates 32 bf16 k-tiles per h-tile (8),
DVE adds b2 PSUM->SBUF f32, DMA out. Weights stay SBUF-resident.
"""

import sys
from contextlib import ExitStack

import numpy as np

for _p in ("/opt/trn_rl_repo",):
    if _p not in sys.path:
        sys.path.insert(0, _p)

import ml_dtypes

import concourse.bacc as bacc
import concourse.tile as tile
from concourse import mybir
from concourse.bass_utils import run_bass_kernel_spmd

BF16 = ml_dtypes.bfloat16
F8 = ml_dtypes.float8_e4m3  # TRN FP8_EXP4: max 240, same bits as OCP below 240

W, E, C, H = 8, 4, 2048, 1024
DFF = 4 * H
N_CORES = 8
P = 128
T = (W // 2) * C          # tokens per core = 8192
KH = H // P               # 8 k-tiles over H
KH8 = 2                   # k-tiles of H done in fp8 DoubleRow (first 256)
KHB = KH - KH8            # 6 bf16 k-tiles
KF = DFF // P             # 32 k-tiles over DFF
NCHUNK = 512
NT = T // NCHUNK          # 16 chunks
SX = 0.25                 # fp8 activation scale (2^-2, exact)
SW = 4.0                  # fp8 weight scale   (2^+2, exact)

_PROG = None              # cached compiled program


def build_program():
    nc = bacc.Bacc("TRN2", target_bir_lowering=False, debug=False,
                   num_devices=N_CORES)
    x8_ap = nc.dram_tensor("x8", [P, KH8, T], mybir.dt.float8e4,
                           kind="ExternalInput").ap()
    xt_ap = nc.dram_tensor("xt", [P, KHB, T], mybir.dt.bfloat16,
                           kind="ExternalInput").ap()
    # weights grouped by OUTPUT tile m (all k-slices of one m are one
    # contiguous DMA), so each m-tile's matmuls unblock independently
    w1dr_ap = nc.dram_tensor("w1dr", [P, KF, KH8, P], mybir.dt.float8e4,
                             kind="ExternalInput").ap()
    w1_ap = nc.dram_tensor("w1", [P, KF, KHB, P], mybir.dt.bfloat16,
                           kind="ExternalInput").ap()
    w2_ap = nc.dram_tensor("w2", [P, KH, KF, P], mybir.dt.bfloat16,
                           kind="ExternalInput").ap()
    b1_ap = nc.dram_tensor("b1", [P, KF], mybir.dt.float32,
                           kind="ExternalInput").ap()
    b2_ap = nc.dram_tensor("b2", [P, KH], mybir.dt.float32,
                           kind="ExternalInput").ap()
    out_ap = nc.dram_tensor("out", [P, KH, T], mybir.dt.float32,
                            kind="ExternalOutput").ap()

    gelu = mybir.ActivationFunctionType.Gelu_apprx_tanh
    DR = mybir.MatmulPerfMode.DoubleRow

    with tile.TileContext(nc) as tc:
        with ExitStack() as ctx:
            wpool = ctx.enter_context(tc.tile_pool(name="weights", bufs=1))
            xpool = ctx.enter_context(tc.tile_pool(name="x", bufs=2))
            x8pool = ctx.enter_context(tc.tile_pool(name="x8", bufs=2))
            gpool = ctx.enter_context(tc.tile_pool(name="g", bufs=1))
            opool = ctx.enter_context(tc.tile_pool(name="o", bufs=4))
            ps1 = ctx.enter_context(tc.tile_pool(name="ps1", bufs=4,
                                                 space="PSUM"))
            ps2 = ctx.enter_context(tc.tile_pool(name="ps2", bufs=4,
                                                 space="PSUM"))

            w1dr_sb = wpool.tile([P, KF, KH8, P], mybir.dt.float8e4,
                                 tag="w1dr")
            w1_sb = wpool.tile([P, KF, KHB, P], mybir.dt.bfloat16, tag="w1")
            w2_sb = wpool.tile([P, KH, KF, P], mybir.dt.bfloat16, tag="w2")
            b1_sb = wpool.tile([P, KF], mybir.dt.float32, tag="b1")
            b2_sb = wpool.tile([P, KH], mybir.dt.float32, tag="b2")
            # DMA issue order = need order: x chunk 0 (split per k-tile
            # so the first matmul fires after ~128KB, not 2MB), biases,
            # then W1 per m-tile; W2 isn't read until GEMM2 of chunk 0
            # (~50us in), so it loads last and overlaps GEMM1.
            # PE warmup: ~16 dummy matmuls with no DMA dependency run
            # while the first x/w1 transfers land, flipping the HAM
            # clock gate to 8/8 before real work starts.
            warm_sb = wpool.tile([P, NCHUNK], mybir.dt.bfloat16, tag="warm")
            nc.vector.memset(warm_sb[:], 0)
            warm_ps = ps1.tile([P, NCHUNK], mybir.dt.float32, tag="ps1",
                               name="warm_ps")
            for _ in range(8):
                nc.tensor.matmul(warm_ps[:], lhsT=warm_sb[:, :P],
                                 rhs=warm_sb[:], start=True, stop=True)

            # Each dma_start costs ~600ns of serial issue time on the Sync
            # queue, so the initial weight load is batched into few large
            # transfers ordered by first-use time; only the m=0 tiles stay
            # separate so chunk 0 starts as early as possible.
            x8_tiles = {}
            x_tiles = {}
            x8_tiles[0] = x8pool.tile([P, KH8, NCHUNK], mybir.dt.float8e4,
                                      tag="x8", name="x8_sb")
            x_tiles[0] = xpool.tile([P, KHB, NCHUNK], mybir.dt.bfloat16,
                                    tag="x", name="x_sb")
            nc.sync.dma_start(x8_tiles[0][:], x8_ap[:, :, 0:NCHUNK])
            nc.sync.dma_start(w1dr_sb[:, 0], w1dr_ap[:, 0])
            nc.sync.dma_start(w1_sb[:, 0], w1_ap[:, 0])
            nc.sync.dma_start(x_tiles[0][:, 0:3, :], xt_ap[:, 0:3, 0:NCHUNK])
            nc.sync.dma_start(x_tiles[0][:, 3:6, :], xt_ap[:, 3:6, 0:NCHUNK])
            nc.sync.dma_start(b1_sb[:], b1_ap[:])
            nc.sync.dma_start(w1dr_sb[:, 1:8], w1dr_ap[:, 1:8])
            nc.sync.dma_start(w1dr_sb[:, 8:KF], w1dr_ap[:, 8:KF])
            for m in range(1, KF):
                nc.sync.dma_start(w1_sb[:, m], w1_ap[:, m])
            nc.sync.dma_start(b2_sb[:], b2_ap[:])
            for m in range(0, KH, 2):
                nc.sync.dma_start(w2_sb[:, m:m + 2], w2_ap[:, m:m + 2])

            for c in range(NT):
                tok = slice(c * NCHUNK, (c + 1) * NCHUNK)
                if c not in x_tiles:
                    x8_tiles[c] = x8pool.tile([P, KH8, NCHUNK],
                                              mybir.dt.float8e4,
                                              tag="x8", name="x8_sb")
                    nc.sync.dma_start(x8_tiles[c][:], x8_ap[:, :, tok])
                    x_tiles[c] = xpool.tile([P, KHB, NCHUNK],
                                            mybir.dt.bfloat16,
                                            tag="x", name="x_sb")
                    nc.sync.dma_start(x_tiles[c][:], xt_ap[:, :, tok])
                x8_sb = x8_tiles.pop(c)
                x_sb = x_tiles.pop(c)

                g_sb = gpool.tile([P, KF, NCHUNK], mybir.dt.bfloat16, tag="g")
                for m in range(KF):
                    pt = ps1.tile([P, NCHUNK], mybir.dt.float32, tag="ps1")
                    # k-tiles 0,1 fused into one fp8 DoubleRow matmul
                    nc.tensor.matmul(
                        pt[:],
                        lhsT=w1dr_sb[:, m],
                        rhs=x8_sb[:],
                        start=True, stop=False, perf_mode=DR)
                    for k in range(KHB):
                        nc.tensor.matmul(
                            pt[:],
                            lhsT=w1_sb[:, m, k, :],
                            rhs=x_sb[:, k, :],
                            start=False, stop=(k == KHB - 1))
                    nc.scalar.activation(g_sb[:, m, :], pt[:], gelu,
                                         bias=b1_sb[:, m:m + 1], scale=1.0)

                for m in range(KH):
                    # The very last output group is split into two half-token
                    # PSUM groups so the first half's bias-add + store DMA
                    # overlap the second half's matmuls, shortening the
                    # end-of-kernel drain.
                    last = (c == NT - 1 and m == KH - 1)
                    splits = 2 if last else 1
                    hw_ = NCHUNK // splits
                    pt2 = ps2.tile([P, NCHUNK], mybir.dt.float32, tag="ps2")
                    o_sb = opool.tile([P, NCHUNK], mybir.dt.float32, tag="o")
                    for s in range(splits):
                        cs = slice(s * hw_, (s + 1) * hw_)
                        ts_ = slice(c * NCHUNK + s * hw_,
                                    c * NCHUNK + (s + 1) * hw_)
                        for k in range(KF):
                            nc.tensor.matmul(
                                pt2[:, cs],
                                lhsT=w2_sb[:, m, k, :],
                                rhs=g_sb[:, k, cs],
                                start=(k == 0), stop=(k == KF - 1))
                        nc.vector.tensor_scalar_add(o_sb[:, cs], pt2[:, cs],
                                                    b2_sb[:, m:m + 1])
                        nc.sync.dma_start(out_ap[:, m, ts_], o_sb[:, cs])

    nc.compile()
    return nc


def _get_prog():
    global _PROG
    if _PROG is None:
        _PROG = build_program()
    return _PROG


def _shard_inputs(inputs, W1, b1, W2, b2):
    inputs = np.asarray(inputs, dtype=np.float32)
    W1 = np.asarray(W1, dtype=np.float32)
    b1 = np.asarray(b1, dtype=np.float32)
    W2 = np.asarray(W2, dtype=np.float32)
    b2 = np.asarray(b2, dtype=np.float32)
    in_maps = []
    for core in range(N_CORES):
        e = core // 2
        wlo = (core % 2) * (W // 2)
        X = np.ascontiguousarray(inputs[wlo:wlo + W // 2, e]).reshape(T, H)
        # fp8 part: columns 0:256, scaled by 2^-2 (exact)
        # [T,256] -> [256,T] -> [2,128,T] -> [128,2,T]
        x8 = np.ascontiguousarray(
            (X[:, :KH8 * P].T * SX).reshape(KH8, P, T).transpose(1, 0, 2)
            .astype(F8))
        # bf16 part: columns 256:1024
        xt = np.ascontiguousarray(
            X[:, KH8 * P:].T.reshape(KHB, P, T).transpose(1, 0, 2)
            .astype(BF16))
        # W1 fp8 rows 0:256 scaled by 2^2: [256,4096] -> [2,128,32,128]
        # [i,p,m,c] -> [p,m,i,c]
        w1dr = np.ascontiguousarray(
            (W1[e][:KH8 * P] * SW).reshape(KH8, P, KF, P)
            .transpose(1, 2, 0, 3).astype(F8))
        # W1 bf16 rows 256:1024: [768,4096] -> [6,128,32,128] -> [p,m,k,c]
        w1 = np.ascontiguousarray(
            W1[e][KH8 * P:].astype(BF16).reshape(KHB, P, KF, P)
            .transpose(1, 2, 0, 3))
        # W2[f,h], f=k*128+p, h=m*128+c -> [p, m, k, c]
        w2 = np.ascontiguousarray(
            W2[e].astype(BF16).reshape(KF, P, KH, P).transpose(1, 2, 0, 3))
        b1c = np.ascontiguousarray(b1[e].reshape(KF, P).T)
        b2c = np.ascontiguousarray(b2[e].reshape(KH, P).T)
        in_maps.append({"x8": x8, "xt": xt, "w1dr": w1dr, "w1": w1,
                        "w2": w2, "b1": b1c, "b2": b2c})
    return in_maps


def _unshard(results):
    out = np.empty((W, E, C, H), dtype=np.float32)
    for core in range(N_CORES):
        e = core // 2
        wlo = (core % 2) * (W // 2)
        dev = results[core]["out"]                      # [P, KH, T]
        Y = dev.transpose(2, 1, 0).reshape(W // 2, C, H)  # [t,m,p] -> [T,H]
        out[wlo:wlo + W // 2, e] = Y
    return out


def run_sharded(in_maps, **kwargs):
    """Compile (cached) + run on cores 0-7; returns BassKernelResults."""
    nc = _get_prog()
    return run_bass_kernel_spmd(nc, in_maps, list(range(N_CORES)), **kwargs)


def kernel(inputs, W1, b1, W2, b2):
    in_maps = _shard_inputs(inputs, W1, b1, W2, b2)
    res = run_sharded(in_maps)
    return _unshard(res.results)
